# revision 1
# baseline (speedup 1.0000x reference)
"""DistMult metapath scoring kernel for Trainium2 (8 NeuronCores).

Math (from the reference): every output group reduces to
    score = emb_h[idx] @ c        with c = K @ s a fixed [d] vector per group
where s is a sum of gathered embedding rows:
    pos0: idx=ei0[0]         s=sum emb_A[ei0[1]]     c=K0@s
    pos1: idx=ei1[0]         s=sum emb_B[ei1[1]]     c=K1@s
    nh0:  idx=nh0.flat       s=sum emb_A[nh0[:,0]]   c=16*K0@s
    nh1:  idx=nh1.flat       s=sum emb_A[nh1[:,0]]   c=16*K1@s
    nt0:  idx=nt0[:,0] (x16) s=sum emb_A[nt0.flat]   c=K0@s
    nt1:  idx=nt1[:,0] (x16) s=sum emb_B[nt1.flat]   c=K1@s

Bulk row gathers use InstDMAGatherAnt (dma_gather): thousands of rows per
instruction, int16 indices wrapped [16, n/16] (replicated to 128 partitions).
Tables are sharded into 25000-row shards (int16 range) with appended zero
rows; indices are bucketed by shard on the host and padded with the zero-row
index (harmless for sums; dot-phase pad scores are dropped by the host
inverse permutation).

ONE SPMD launch on 8 cores:
  - per-core partial sums of the 6 groups (bucketed dma_gather + DVE
    accumulate + ones-matmul cross-partition reduce)
  - in-kernel AllReduce of the [6,128] partials
  - head: c_g = K_{g%2} @ s_g on TensorE, broadcast across partitions
  - score segments: bucketed dma_gather of embedding rows (independent of
    the sums, so these transfers overlap the whole reduction), DVE
    mul+reduce dot with c, chunk scores stored p-major
Host glue: index bucketing/padding (layout only), inverse-permutation
unshard of the scores (the nt x16 expansion folds into the same take-map).
"""

import sys
from contextlib import ExitStack

import numpy as np

sys.path.insert(0, "/opt/trn_rl_repo")

import concourse.bass as bass
from concourse import bacc, mybir
from concourse.bass_utils import run_bass_kernel_spmd
from concourse.masks import make_identity
from concourse.tile import TileContext

D = 128
E = 50000
S = 16
NA = 100000
NB = 50000
NCORES = 8

EC = E // NCORES        # 6250 edge items per core
FC = (E * S) // NCORES  # 100000 flat neg items per core

SH = 25000              # table rows per shard
SHP = 25024             # shard rows incl. zero pad rows
ZIDX = 25000            # local index of a guaranteed-zero row
NSH_A, NSH_B = 4, 2

F32 = mybir.dt.float32
I16 = mybir.dt.int16
X = mybir.AxisListType.X
ADD = mybir.AluOpType.add

# chunk lists per bucket capacity (each chunk = one dma_gather instruction)
CH_2048 = [2048]
CH_4096 = [4096]
CH_26624 = [4096] * 6 + [2048]
CH_52224 = [4096] * 12 + [2048, 2048]

# sum groups: (name, items/core, table, per-shard chunk list)
L1_GROUPS = [
    ("s0", EC, "A", CH_2048),
    ("s1", EC, "B", CH_4096),
    ("h0", EC, "A", CH_2048),
    ("h1", EC, "A", CH_2048),
    ("t0", FC, "A", CH_26624),
    ("t1", FC, "B", CH_52224),
]

# score segments: (name, items/core, table, c column, per-shard chunk list)
L2_SEGS = [
    ("pos0", EC, "A", 0, CH_2048),
    ("pos1", EC, "A", 1, CH_2048),
    ("nh0", FC, "A", 2, CH_26624),
    ("nh1", FC, "A", 3, CH_26624),
    ("nt0", EC, "A", 4, CH_2048),   # bases; x16 expand via host take-map
    ("nt1", EC, "B", 5, CH_4096),
]


def _nsh(t):
    return NSH_A if t == "A" else NSH_B


def _cap(chunks):
    return sum(chunks)


def build_fused(repeat: int = 1, fake_cc: bool = False) -> bass.Bass:
    nc = bacc.Bacc(None, target_bir_lowering=False)
    tabA = nc.dram_tensor("tabA", [NSH_A, SHP, D], F32, kind="ExternalInput")
    tabB = nc.dram_tensor("tabB", [NSH_B, SHP, D], F32, kind="ExternalInput")
    rel = nc.dram_tensor("rel", [2, D, D], F32, kind="ExternalInput")
    xin, outs = {}, {}
    for name, L, t, chunks in L1_GROUPS:
        W = _cap(chunks) * _nsh(t) // 16
        xin[name] = nc.dram_tensor("x_" + name, [128, W], I16, kind="ExternalInput")
    for name, L, t, cc, chunks in L2_SEGS:
        cap = _cap(chunks) * _nsh(t)
        xin[name] = nc.dram_tensor(
            "xs_" + name, [128, cap // 16], I16, kind="ExternalInput"
        )
        outs[name] = nc.dram_tensor("o_" + name, [cap], F32, kind="ExternalOutput")
    cc_in = [
        (
            nc.dram_tensor(f"cc_ina{r}", [4, D], F32),
            nc.dram_tensor(f"cc_inb{r}", [2, D], F32),
        )
        for r in range(repeat)
    ]
    cc_out = [
        (
            nc.dram_tensor(f"cc_outa{r}", [4, D], F32, addr_space="Shared"),
            nc.dram_tensor(f"cc_outb{r}", [2, D], F32, addr_space="Shared"),
        )
        for r in range(repeat)
    ]

    with ExitStack() as ctx:
        tc = ctx.enter_context(TileContext(nc))
        sing = ctx.enter_context(tc.tile_pool(name="sing", bufs=1))
        gb = ctx.enter_context(tc.tile_pool(name="gbuf", bufs=7))
        ib = ctx.enter_context(tc.tile_pool(name="ibuf", bufs=1))
        scp = ctx.enter_context(tc.tile_pool(name="sc", bufs=3))
        ppA = ctx.enter_context(tc.tile_pool(name="ppA", bufs=1, space="PSUM"))
        ppB = ctx.enter_context(tc.tile_pool(name="ppB", bufs=1, space="PSUM"))
        ppC = ctx.enter_context(tc.tile_pool(name="ppC", bufs=1, space="PSUM"))
        ppD = ctx.enter_context(tc.tile_pool(name="ppD", bufs=1, space="PSUM"))

        ident = sing.tile([128, 128], F32)
        make_identity(nc, ident[:, :])
        ones = sing.tile([128, 1], F32)
        nc.vector.memset(ones, 1.0)
        ones1 = sing.tile([1, 128], F32)
        nc.vector.memset(ones1, 1.0)
        for rep in range(repeat):
            _fused_body(
                nc, tc, sing, gb, ib, scp, ppA, ppB, ppC, ppD,
                ident, ones, ones1,
                tabA, tabB, rel, xin, outs, cc_in[rep], cc_out[rep],
                fake_cc, rep,
            )
    nc.compile()
    return nc


def _gather_chunk(nc, gb, tab, s, it, col, n):
    bt = gb.tile([128, 4096], F32, tag="g")
    nc.gpsimd.dma_gather(
        out_ap=bt[:, :n].rearrange("p (c e) -> p c e", e=D),
        in_ap=tab[s],
        idxs_ap=it[:, col : col + n // 16],
        num_idxs=n,
        num_idxs_reg=n,
        elem_size=D,
        single_packet=False,
    )
    return bt


def _fused_body(
    nc, tc, sing, gb, ib, scp, ppA, ppB, ppC, ppD,
    ident, ones, ones1,
    tabA, tabB, rel, xin, outs, cc_in, cc_out, fake_cc, rep,
):
    F32R = mybir.dt.float32r
    MULT = mybir.AluOpType.mult
    cc_in_a, cc_in_b = cc_in
    cc_out_a, cc_out_b = cc_out
    tabs = {"A": tabA, "B": tabB}

    # per-group metadata
    g_small = [g for g in L1_GROUPS if g[0] in ("h0", "h1", "s0", "s1")]
    g_big = {g[0]: g for g in L1_GROUPS if g[0] in ("t0", "t1")}
    segs = {s[0]: s for s in L2_SEGS}

    idx_tiles = {}

    def load_idx(name, dram, chunks, nsh):
        W = _cap(chunks) * nsh // 16
        it = ib.tile([128, W], I16, tag="idx" + name)
        nc.sync.dma_start(out=it[:, :], in_=dram[:, :])
        idx_tiles[name] = it
        return it

    # matmul-accumulate a gathered chunk into a [1,512] psum accumulator
    def pe_acc(bt, n, accps, st):
        for q in range(n // 512):
            nc.tensor.matmul(
                out=accps[:, :],
                lhsT=ones[:, :],
                rhs=bt[:, q * 512 : (q + 1) * 512],
                start=(st["i"] == 0),
                stop=(st["i"] == st["n"] - 1),
                skip_group_check=True,
            )
            st["i"] += 1

    def dot_chunk(name, cc, bt, n, base):
        bc = CB[cc][:, :]
        bc_ap = bass.AP(
            tensor=bc.tensor, offset=bc.offset,
            ap=[bc.ap[0], [0, n // 128], [1, 128]],
        )
        nc.vector.tensor_tensor(
            out=bt[:, :n], in0=bt[:, :n], in1=bc_ap, op=MULT
        )
        sc = scp.tile([128, 32], F32, tag="s")
        nc.vector.tensor_reduce(
            out=sc[:, : n // 128],
            in_=bt[:, :n].rearrange("p (c d) -> p c d", d=D),
            axis=X,
            op=ADD,
        )
        nc.sync.dma_start(
            out=outs[name][base : base + n].rearrange("(p c) -> p c", p=128),
            in_=sc[:, : n // 128],
        )

    def reduce_acc(accps, dst_ap):
        nc.vector.tensor_reduce(
            out=dst_ap,
            in_=accps[:, :].rearrange("p (c d) -> p d c", d=D),
            axis=X,
            op=ADD,
        )

    # ---------------- phase 1: small sum groups -> AllReduce #1
    pvec_a = sing.tile([1, 4 * D], F32, tag=f"pva0")
    # order within cc_in_a rows: [s0, s1, h0, h1] = c columns 0..3
    small_order = ["s0", "s1", "h0", "h1"]
    for gi, name in enumerate(small_order):
        _, L, t, chunks = next(g for g in g_small if g[0] == name)
        tab = tabs[t]
        nsh = _nsh(t)
        it = load_idx(name, xin[name], chunks, nsh)
        accps = ppA.tile([1, 512], F32, tag="accS")
        st = {"i": 0, "n": sum(n // 512 for n in chunks) * nsh}
        col = 0
        for s in range(nsh):
            for n in chunks:
                bt = _gather_chunk(nc, gb, tab, s, it, col, n)
                pe_acc(bt, n, accps, st)
                col += n // 16
        reduce_acc(accps, pvec_a[:, gi * D : (gi + 1) * D])
    nc.sync.dma_start(
        out=cc_in_a[:, :].rearrange("a b -> (a b)")[None, :], in_=pvec_a[:, :]
    )
    if fake_cc:
        nc.gpsimd.dma_start(out=cc_out_a[:, :], in_=cc_in_a[:, :])
    else:
        nc.gpsimd.collective_compute(
            "AllReduce",
            mybir.AluOpType.add,
            replica_groups=[list(range(NCORES))],
            ins=[cc_in_a[:, :]],
            outs=[cc_out_a[:, :]],
        )

    # ---------------- head A: c_0..c_3 and broadcast tiles
    CB = [None] * 6

    KT = []
    for m in range(2):
        kin = sing.tile([128, 128], F32, tag=f"kin{m}")
        nc.sync.dma_start(out=kin[:, :], in_=rel[m, :, :])
        kt_ps = ppC.tile([128, 128], F32, tag="ktp")
        nc.tensor.transpose(out=kt_ps[:, :], in_=kin[:, :], identity=ident[:, :])
        kt = sing.tile([128, 128], F32, tag=f"kt{m}")
        nc.vector.tensor_copy(kt[:, :], kt_ps[:, :])
        KT.append(kt)

    def head(cc_out_t, nrows, cols, scale16):
        """cols: list of global c columns; cc_out_t rows map 1:1 to cols."""
        sred = sing.tile([nrows, D], F32, tag=f"sred{len(cols)}")
        nc.sync.dma_start(out=sred[:, :], in_=cc_out_t[:, :])
        sT_ps = ppB.tile([128, 6], F32, tag="sT")
        nc.tensor.transpose(
            out=sT_ps[:, :nrows], in_=sred[:, :], identity=ident[:nrows, :nrows]
        )
        sT = sing.tile([128, 6], F32, tag=f"sT{len(cols)}")
        nc.vector.tensor_copy(sT[:, :nrows], sT_ps[:, :nrows])
        c_ps = ppB.tile([128, 6], F32, tag="c")
        for j, g in enumerate(cols):
            nc.tensor.matmul(
                out=c_ps[:, j : j + 1],
                lhsT=KT[g % 2][:, :],
                rhs=sT[:, j : j + 1],
                start=True,
                stop=True,
            )
        c_sb = sing.tile([128, 6], F32, tag=f"csb{len(cols)}")
        for j, g in enumerate(cols):
            if g in scale16:
                nc.vector.tensor_scalar_mul(
                    c_sb[:, j : j + 1], c_ps[:, j : j + 1], float(S)
                )
            else:
                nc.vector.tensor_copy(c_sb[:, j : j + 1], c_ps[:, j : j + 1])
        for j, g in enumerate(cols):
            ct_ps = ppD.tile([1, 128], F32, tag="ctp")
            nc.tensor.transpose(
                out=ct_ps[:, :], in_=c_sb[:, j : j + 1], identity=ident[:, :]
            )
            ct1 = sing.tile([1, 128], F32, tag=f"ct{g}")
            nc.vector.tensor_copy(ct1[:, :], ct_ps[:, :])
            cb_ps = ppD.tile([128, 128], F32, tag="cbp")
            nc.tensor.matmul(
                out=cb_ps[:, :], lhsT=ones1[:, :], rhs=ct1[:, :],
                start=True, stop=True,
            )
            cb = sing.tile([128, 128], F32, tag=f"cb{g}")
            nc.vector.tensor_copy(cb[:, :], cb_ps[:, :])
            CB[g] = cb

    head(cc_out_a, 4, [0, 1, 2, 3], scale16={2, 3})

    # ---------------- phase 2: interleave big sums (t0,t1) with nh dots
    pvec_b = sing.tile([1, 2 * D], F32, tag=f"pvb0")
    streams = []  # (kind, name, tab, shard, n, col, base, acc/cc, state)
    big_states = {}
    for name in ("t0", "t1"):
        _, L, t, chunks = g_big[name]
        it = load_idx(name, xin[name], chunks, _nsh(t))
        accps = ppA.tile([1, 512], F32, tag="acc" + name)
        st = {"i": 0, "n": sum(n // 512 for n in chunks) * _nsh(t)}
        big_states[name] = (accps, st)
        lst = []
        col = 0
        for s in range(_nsh(t)):
            for n in chunks:
                lst.append(("sum", name, tabs[t], s, n, col, 0))
                col += n // 16
        streams.append(lst)
    for name in ("nh0", "nh1"):
        _, L, t, cc, chunks = segs[name]
        it = load_idx(name, xin[name], chunks, _nsh(t))
        lst = []
        col = 0
        base = 0
        for s in range(_nsh(t)):
            for n in chunks:
                lst.append(("dot", name, tabs[t], s, n, col, base))
                col += n // 16
                base += n
        streams.append(lst)
    # round-robin interleave
    mi = 0
    while any(streams):
        lst = streams[mi % len(streams)]
        mi += 1
        if not lst:
            continue
        kind, name, tab, s, n, col, base = lst.pop(0)
        it = idx_tiles[name]
        bt = _gather_chunk(nc, gb, tab, s, it, col, n)
        if kind == "sum":
            accps, st = big_states[name]
            pe_acc(bt, n, accps, st)
        else:
            cc = segs[name][3]
            dot_chunk(name, cc, bt, n, base)

    reduce_acc(big_states["t0"][0], pvec_b[:, 0:D])
    reduce_acc(big_states["t1"][0], pvec_b[:, D : 2 * D])
    nc.sync.dma_start(
        out=cc_in_b[:, :].rearrange("a b -> (a b)")[None, :], in_=pvec_b[:, :]
    )
    if fake_cc:
        nc.gpsimd.dma_start(out=cc_out_b[:, :], in_=cc_in_b[:, :])
    else:
        nc.gpsimd.collective_compute(
            "AllReduce",
            mybir.AluOpType.add,
            replica_groups=[list(range(NCORES))],
            ins=[cc_in_b[:, :]],
            outs=[cc_out_b[:, :]],
        )
    head(cc_out_b, 2, [4, 5], scale16=set())

    # ---------------- phase 3: pos and nt segments
    for name in ("pos0", "pos1", "nt0", "nt1"):
        _, L, t, cc, chunks = segs[name]
        tab = tabs[t]
        nsh = _nsh(t)
        it = load_idx(name, xin[name], chunks, nsh)
        col = 0
        base = 0
        for s in range(nsh):
            for n in chunks:
                bt = _gather_chunk(nc, gb, tab, s, it, col, n)
                dot_chunk(name, segs[name][3], bt, n, base)
                col += n // 16
                base += n


_CACHE = {}


def _programs():
    if "p" not in _CACHE:
        _CACHE["p"] = build_fused()
    return _CACHE["p"]


# ---------------------------------------------------------------- host glue


def _shard_tables(emb_A, emb_B):
    tabA = np.zeros((NSH_A, SHP, D), np.float32)
    for s in range(NSH_A):
        tabA[s, :SH] = emb_A[s * SH : (s + 1) * SH]
    tabB = np.zeros((NSH_B, SHP, D), np.float32)
    for s in range(NSH_B):
        tabB[s, :SH] = emb_B[s * SH : (s + 1) * SH]
    return tabA, tabB


def _wrap16(stream):
    """[L] int -> [128, L//16] int16 (wrapped in 16 partitions, replicated)."""
    L = stream.shape[0]
    w = stream.reshape(L // 16, 16).T.astype(np.int16)  # [16, L/16]
    return np.tile(w, (8, 1))


def _bucketize(idx, nsh, cap):
    """Bucket by shard, pad each bucket to cap with ZIDX.

    Returns (stream [nsh*cap] local indices, qpos [len(idx)]: stream position
    of each original element)."""
    L = idx.shape[0]
    stream = np.full(nsh * cap, ZIDX, np.int64)
    qpos = np.empty(L, np.int64)
    for s in range(nsh):
        m = (idx >= s * SH) & (idx < (s + 1) * SH)
        cnt = int(m.sum())
        assert cnt <= cap, f"bucket overflow: {cnt} > {cap}"
        stream[s * cap : s * cap + cnt] = idx[m] - s * SH
        qpos[m] = s * cap + np.arange(cnt)
    return stream, qpos


def _chunk_pos_map(chunks, nsh):
    """Stream position q -> stored DRAM position.

    The device stores each chunk of size n as a [128, n/128] tile written
    p-major (DRAM[base + p*(n/128) + c]), where in-chunk index i = c*128+p."""
    cap = _cap(chunks)
    pos = np.empty(nsh * cap, np.int64)
    base = 0
    qb = 0
    for s in range(nsh):
        for n in chunks:
            i = np.arange(n)
            pos[qb : qb + n] = base + (i % 128) * (n // 128) + i // 128
            base += n
            qb += n
    return pos


def _build_inputs(emb_A, emb_B, rel_emb, ei0, ei1, nh0, nh1, nt0, nt1):
    """Per-core in_maps + per-core per-segment take maps (device DRAM order
    -> original order, with the nt x16 expansion folded in)."""
    tabA, tabB = _shard_tables(emb_A, emb_B)
    l1_idx = {
        "s0": ei0[1], "s1": ei1[1], "h0": nh0[:, 0], "h1": nh1[:, 0],
        "t0": nt0.reshape(-1), "t1": nt1.reshape(-1),
    }
    l2_idx = {
        "pos0": ei0[0], "pos1": ei1[0],
        "nh0": nh0.reshape(-1), "nh1": nh1.reshape(-1),
        "nt0": nt0[:, 0], "nt1": nt1[:, 0],
    }
    in_maps, take_maps = [], []
    for k in range(NCORES):
        m = {"tabA": tabA, "tabB": tabB, "rel": rel_emb}
        for name, L, t, chunks in L1_GROUPS:
            arr = l1_idx[name]
            per = arr.shape[0] // NCORES
            stream, _ = _bucketize(
                arr[k * per : (k + 1) * per], _nsh(t), _cap(chunks)
            )
            m["x_" + name] = _wrap16(stream)
        tm = {}
        for name, L, t, cc, chunks in L2_SEGS:
            arr = l2_idx[name]
            per = arr.shape[0] // NCORES
            stream, qpos = _bucketize(
                arr[k * per : (k + 1) * per], _nsh(t), _cap(chunks)
            )
            m["xs_" + name] = _wrap16(stream)
            pos = _chunk_pos_map(chunks, _nsh(t))[qpos]
            if name.startswith("nt"):
                pos = np.repeat(pos, S)  # x16 expansion inside the take map
            tm[name] = pos
        in_maps.append(m)
        take_maps.append(tm)
    return in_maps, take_maps


def kernel(
    emb_A,
    emb_B,
    rel_emb,
    edge_index_m0,
    edge_index_m1,
    neg_head_m0,
    neg_head_m1,
    neg_tail_m0,
    neg_tail_m1,
    _results=None,
):
    emb_A = np.ascontiguousarray(np.asarray(emb_A, dtype=np.float32))
    emb_B = np.ascontiguousarray(np.asarray(emb_B, dtype=np.float32))
    rel_emb = np.ascontiguousarray(np.asarray(rel_emb, dtype=np.float32))
    ei0 = np.asarray(edge_index_m0, dtype=np.int64)
    ei1 = np.asarray(edge_index_m1, dtype=np.int64)
    nh0 = np.asarray(neg_head_m0, dtype=np.int64)
    nh1 = np.asarray(neg_head_m1, dtype=np.int64)
    nt0 = np.asarray(neg_tail_m0, dtype=np.int64)
    nt1 = np.asarray(neg_tail_m1, dtype=np.int64)

    prog = _programs()
    cores = list(range(NCORES))
    in_maps, take_maps = _build_inputs(
        emb_A, emb_B, rel_emb, ei0, ei1, nh0, nh1, nt0, nt1
    )
    r = run_bass_kernel_spmd(prog, in_maps, cores)

    segs = {}
    for name, L, t, cc, chunks in L2_SEGS:
        segs[name] = np.concatenate(
            [r.results[k]["o_" + name][take_maps[k][name]] for k in cores]
        )
    if _results is not None:
        _results.append(r)
    return np.concatenate(
        [segs["pos0"], segs["pos1"], segs["nh0"], segs["nh1"],
         segs["nt0"], segs["nt1"]]
    )



# revision 6
# speedup vs baseline: 25.0955x; 25.0955x over previous
"""DistMult metapath scoring kernel for Trainium2 (8 NeuronCores).

Math (from the reference): every output element is emb_h[idx] @ c_g where
c_g = K_{g%2} @ s_g is one of six fixed [d] vectors, and each s_g is a sum
of embedding rows over an index list:
    pos0: emb_A[ei0[0]] . c0   c0 = K0 @ sum emb_A[ei0[1]]
    pos1: emb_A[ei1[0]] . c1   c1 = K1 @ sum emb_B[ei1[1]]
    nh0:  emb_A[nh0.flat] . c2 c2 = 16*K0 @ sum emb_A[nh0[:,0]]
    nh1:  emb_A[nh1.flat] . c3 c3 = 16*K1 @ sum emb_A[nh1[:,0]]
    nt0:  emb_A[nt0[:,0]] . c4 (x16)  c4 = K0 @ sum emb_A[nt0.flat]
    nt1:  emb_B[nt1[:,0]] . c5 (x16)  c5 = K1 @ sum emb_B[nt1.flat]

So instead of gathering ~450k embedding rows per core (the old kernel's
dma_gather bottleneck: GpSimd 97% busy generating descriptors), the device
computes per-NODE score tables with dense matmuls over table shards:
    P_A = emb_A @ [c0 c1 c2 c3 c4]   (each core: its 1/8 of the rows)
    P_B = emb_B @ [c5]
and the host assembles the per-edge outputs by pure indexing (same class of
host glue as the old kernel's take-maps / x16 expansion).

The row sums s_g become matmuls against histograms of the index lists
(h_g[n] = multiplicity of node n), computed host-side from the int index
tensors — index-derived preprocessing only, all float math stays on device.

Device pipeline per core (table-parallel over node rows):
  1. DMA in: table shard in two layouts (row-major wrapped for stage 1,
     transposed for stage 2), histograms, rel kernels.
  2. Stage 1: s-partials = shard^T @ h via PE accumulation over [128,128]
     tiles; one AllReduce of the [128,6] partials.
  3. Head: c_g = K_{g%2} @ s_g on PE (K transposed on PE), x16 for nh.
  4. Stage 2: P^T chunks = C^T @ shard^T on PE (f32r, 512-wide), DMA out.
"""

import sys
from contextlib import ExitStack

import numpy as np

sys.path.insert(0, "/opt/trn_rl_repo")

import concourse.bass as bass
from concourse import bacc, mybir
from concourse.bass_utils import run_bass_kernel_spmd
from concourse.masks import make_identity
from concourse.tile import TileContext

D = 128
E = 50000
S = 16
NA = 100000
NB = 50000
NCORES = 8

SH_A = NA // NCORES       # 12500 A-rows per core
SH_B = NB // NCORES       # 6250 B-rows per core
TA = (SH_A + 127) // 128  # 98 tiles
TB = (SH_B + 127) // 128  # 49 tiles
RA = TA * 128             # 12544 padded rows
RB = TB * 128             # 6272
GA = 4                    # A-table sum groups: [s0, h0, t0, h1]
GB = 2                    # B-table sum groups: [s1, t1]
CW = 512                  # stage-2 chunk width (one PSUM bank of fp32)
RM_CHUNKS_A = 4           # pipelined row-major loads
RM_CHUNKS_B = 2

F32 = mybir.dt.float32
F32R = mybir.dt.float32r


def _chunks(total_tiles, n):
    base = total_tiles // n
    rem = total_tiles % n
    return [base + (1 if i < rem else 0) for i in range(n)]


def build() -> bass.Bass:
    nc = bacc.Bacc(None, target_bir_lowering=False)
    rmA = nc.dram_tensor("rmA", [128, RA], F32, kind="ExternalInput")
    rmB = nc.dram_tensor("rmB", [128, RB], F32, kind="ExternalInput")
    atA = nc.dram_tensor("atA", [128, RA], F32R, kind="ExternalInput")
    atB = nc.dram_tensor("atB", [128, RB], F32R, kind="ExternalInput")
    hA = nc.dram_tensor("hA", [128, TA * GA], F32, kind="ExternalInput")
    hB = nc.dram_tensor("hB", [128, TB * GB], F32, kind="ExternalInput")
    rel = nc.dram_tensor("rel", [2, D, D], F32, kind="ExternalInput")
    o_PA = nc.dram_tensor("o_PA", [5, RA], F32, kind="ExternalOutput")
    o_PB = nc.dram_tensor("o_PB", [1, RB], F32, kind="ExternalOutput")
    cc_in = nc.dram_tensor("cc_in", [128, 6], F32)
    cc_out = nc.dram_tensor("cc_out", [128, 6], F32, addr_space="Shared")

    with ExitStack() as ctx:
        tc = ctx.enter_context(TileContext(nc))
        sing = ctx.enter_context(tc.tile_pool(name="sing", bufs=1))
        scp = ctx.enter_context(tc.tile_pool(name="scp", bufs=4))
        ppK = ctx.enter_context(tc.tile_pool(name="ppK", bufs=2, space="PSUM"))
        ppS = ctx.enter_context(tc.tile_pool(name="ppS", bufs=1, space="PSUM"))
        ppP = ctx.enter_context(tc.tile_pool(name="ppP", bufs=3, space="PSUM"))

        ident = sing.tile([128, 128], F32)
        make_identity(nc, ident[:, :])

        # ---- input DMAs (sync queue, in priority order) ----
        kin = []
        for m in range(2):
            ki = sing.tile([128, 128], F32, tag=f"kin{m}")
            nc.sync.dma_start(out=ki[:, :], in_=rel[m, :, :])
            kin.append(ki)
        hA_t = sing.tile([128, TA * GA], F32, tag="hA")
        nc.sync.dma_start(out=hA_t[:, :], in_=hA[:, :])
        hB_t = sing.tile([128, TB * GB], F32, tag="hB")
        nc.sync.dma_start(out=hB_t[:, :], in_=hB[:, :])

        rmA_tiles = []  # (tile, ntiles)
        off = 0
        for i, nt in enumerate(_chunks(TA, RM_CHUNKS_A)):
            t = sing.tile([128, nt * 128], F32, tag=f"rmA{i}")
            nc.sync.dma_start(out=t[:, :], in_=rmA[:, off : off + nt * 128])
            rmA_tiles.append((t, nt))
            off += nt * 128
        rmB_tiles = []
        off = 0
        for i, nt in enumerate(_chunks(TB, RM_CHUNKS_B)):
            t = sing.tile([128, nt * 128], F32, tag=f"rmB{i}")
            nc.sync.dma_start(out=t[:, :], in_=rmB[:, off : off + nt * 128])
            rmB_tiles.append((t, nt))
            off += nt * 128

        atA_t = sing.tile([128, RA], F32R, tag="atA")
        nc.sync.dma_start(out=atA_t[:, :], in_=atA[:, :])
        atB_t = sing.tile([128, RB], F32R, tag="atB")
        nc.sync.dma_start(out=atB_t[:, :], in_=atB[:, :])

        # ---- K transposes (PE, early) ----
        KT = []
        for m in range(2):
            kt_ps = ppK.tile([128, 128], F32, tag="kt")
            nc.tensor.transpose(out=kt_ps[:, :], in_=kin[m][:, :], identity=ident[:, :])
            kt = sing.tile([128, 128], F32, tag=f"kt{m}")
            nc.vector.tensor_copy(kt[:, :], kt_ps[:, :])
            KT.append(kt)

        # ---- stage 1: s-partials = shard^T @ h ----
        s_psA = ppS.tile([128, GA], F32, tag="sA")
        gt = 0
        for t, nt in rmA_tiles:
            for i in range(nt):
                nc.tensor.matmul(
                    out=s_psA[:, :],
                    lhsT=t[:, i * 128 : (i + 1) * 128],
                    rhs=hA_t[:, gt * GA : (gt + 1) * GA],
                    start=(gt == 0),
                    stop=(gt == TA - 1),
                )
                gt += 1
        s_psB = ppS.tile([128, GB], F32, tag="sB")
        gt = 0
        for t, nt in rmB_tiles:
            for i in range(nt):
                nc.tensor.matmul(
                    out=s_psB[:, :],
                    lhsT=t[:, i * 128 : (i + 1) * 128],
                    rhs=hB_t[:, gt * GB : (gt + 1) * GB],
                    start=(gt == 0),
                    stop=(gt == TB - 1),
                )
                gt += 1

        # cols of s_sb: [s0, h0, t0, h1, s1, t1]; K0 -> 0:3, K1 -> 3:6
        s_sb = sing.tile([128, 6], F32, tag="s_sb")
        nc.vector.tensor_copy(s_sb[:, 0:GA], s_psA[:, :])
        nc.vector.tensor_copy(s_sb[:, GA : GA + GB], s_psB[:, :])
        nc.scalar.dma_start(out=cc_in[:, :], in_=s_sb[:, :])
        nc.gpsimd.collective_compute(
            "AllReduce",
            mybir.AluOpType.add,
            replica_groups=[list(range(NCORES))],
            ins=[cc_in[:, :]],
            outs=[cc_out[:, :]],
        )
        s_all = sing.tile([128, 6], F32, tag="s_all")
        nc.scalar.dma_start(out=s_all[:, :], in_=cc_out[:, :])

        # ---- head: c_ps cols = [K0s0, K0h0, K0t0, K1h1, K1s1, K1t1] ----
        c_ps = ppS.tile([128, 6], F32, tag="c")
        nc.tensor.matmul(
            out=c_ps[:, 0:3], lhsT=KT[0][:, :], rhs=s_all[:, 0:3],
            start=True, stop=True,
        )
        nc.tensor.matmul(
            out=c_ps[:, 3:6], lhsT=KT[1][:, :], rhs=s_all[:, 3:6],
            start=True, stop=True,
        )
        # C_A cols: [pos0=cp0, nh0=16*cp1, nt0=cp2, pos1=cp4, nh1=16*cp3]
        C_A = sing.tile([128, 5], F32R, tag="C_A")
        C_B = sing.tile([128, 1], F32R, tag="C_B")
        nc.vector.tensor_copy(C_A[:, 0:3], c_ps[:, 0:3])
        nc.vector.tensor_copy(C_A[:, 3:4], c_ps[:, 4:5])
        nc.vector.tensor_scalar_mul(C_A[:, 4:5], c_ps[:, 3:4], float(S))
        nc.vector.tensor_scalar_mul(C_A[:, 1:2], C_A[:, 1:2], float(S))
        nc.vector.tensor_copy(C_B[:, 0:1], c_ps[:, 5:6])

        # ---- stage 2: P^T chunks = C^T @ shard^T (f32r full-rate) ----
        def stage2(C, ncols, at_t, width, o_dram):
            for off in range(0, width, CW):
                w = min(CW, width - off)
                pt = ppP.tile([5, CW], F32, tag="p")
                nc.tensor.matmul(
                    out=pt[:ncols, :w],
                    lhsT=C[:, :ncols],
                    rhs=at_t[:, off : off + w],
                    start=True,
                    stop=True,
                )
                st = scp.tile([5, CW], F32, tag="st")
                nc.vector.tensor_copy(st[:ncols, :w], pt[:ncols, :w])
                nc.scalar.dma_start(
                    out=o_dram[:, off : off + w], in_=st[:ncols, :w]
                )

        stage2(C_A, 5, atA_t, RA, o_PA)
        stage2(C_B, 1, atB_t, RB, o_PB)

    nc.compile()
    return nc


_CACHE = {}


def _program():
    if "p" not in _CACHE:
        _CACHE["p"] = build()
    return _CACHE["p"]


# ---------------------------------------------------------------- host glue


def _wrap_rm(shard, ntiles, ncols):
    """[ntiles*128, ncols] -> [128, ntiles*ncols] with [p, t*ncols+c] =
    shard[t*128+p, c] (tile-wrapped row-major layout)."""
    return np.ascontiguousarray(
        shard.reshape(ntiles, 128, ncols).transpose(1, 0, 2).reshape(128, -1)
    )


def kernel(
    emb_A,
    emb_B,
    rel_emb,
    edge_index_m0,
    edge_index_m1,
    neg_head_m0,
    neg_head_m1,
    neg_tail_m0,
    neg_tail_m1,
    _results=None,
):
    emb_A = np.ascontiguousarray(np.asarray(emb_A, dtype=np.float32))
    emb_B = np.ascontiguousarray(np.asarray(emb_B, dtype=np.float32))
    rel_emb = np.ascontiguousarray(np.asarray(rel_emb, dtype=np.float32))
    ei0 = np.asarray(edge_index_m0, dtype=np.int64)
    ei1 = np.asarray(edge_index_m1, dtype=np.int64)
    nh0 = np.asarray(neg_head_m0, dtype=np.int64)
    nh1 = np.asarray(neg_head_m1, dtype=np.int64)
    nt0 = np.asarray(neg_tail_m0, dtype=np.int64)
    nt1 = np.asarray(neg_tail_m1, dtype=np.int64)

    # index-derived histograms (multiplicity of each node in each sum group)
    histA = np.zeros((NA, GA), np.float32)  # cols [s0, h0, t0, h1]
    histA[:, 0] = np.bincount(ei0[1], minlength=NA)
    histA[:, 1] = np.bincount(nh0[:, 0], minlength=NA)
    histA[:, 2] = np.bincount(nt0.reshape(-1), minlength=NA)
    histA[:, 3] = np.bincount(nh1[:, 0], minlength=NA)
    histB = np.zeros((NB, GB), np.float32)  # cols [s1, t1]
    histB[:, 0] = np.bincount(ei1[1], minlength=NB)
    histB[:, 1] = np.bincount(nt1.reshape(-1), minlength=NB)

    in_maps = []
    for k in range(NCORES):
        a_sh = np.zeros((RA, D), np.float32)
        a_sh[:SH_A] = emb_A[k * SH_A : (k + 1) * SH_A]
        b_sh = np.zeros((RB, D), np.float32)
        b_sh[:SH_B] = emb_B[k * SH_B : (k + 1) * SH_B]
        ha = np.zeros((RA, GA), np.float32)
        ha[:SH_A] = histA[k * SH_A : (k + 1) * SH_A]
        hb = np.zeros((RB, GB), np.float32)
        hb[:SH_B] = histB[k * SH_B : (k + 1) * SH_B]
        in_maps.append(
            {
                "rmA": _wrap_rm(a_sh, TA, D),
                "rmB": _wrap_rm(b_sh, TB, D),
                "atA": np.ascontiguousarray(a_sh.T),
                "atB": np.ascontiguousarray(b_sh.T),
                "hA": _wrap_rm(ha, TA, GA),
                "hB": _wrap_rm(hb, TB, GB),
                "rel": rel_emb,
            }
        )

    prog = _program()
    cores = list(range(NCORES))
    r = run_bass_kernel_spmd(prog, in_maps, cores)
    if _results is not None:
        _results.append(r)

    # per-node score tables: PA cols [pos0, nh0, nt0, pos1, nh1]; PB [nt1]
    PA = np.empty((NA, 5), np.float32)
    PB = np.empty((NB, 1), np.float32)
    for k in cores:
        PA[k * SH_A : (k + 1) * SH_A] = r.results[k]["o_PA"][:, :SH_A].T
        PB[k * SH_B : (k + 1) * SH_B] = r.results[k]["o_PB"][:, :SH_B].T

    pos0 = PA[ei0[0], 0]
    pos1 = PA[ei1[0], 3]
    nh0_s = PA[nh0.reshape(-1), 1]
    nh1_s = PA[nh1.reshape(-1), 4]
    nt0_s = np.repeat(PA[nt0[:, 0], 2], S)
    nt1_s = np.repeat(PB[nt1[:, 0], 0], S)
    return np.concatenate([pos0, pos1, nh0_s, nh1_s, nt0_s, nt1_s])


# revision 15
# speedup vs baseline: 31.3940x; 1.2510x over previous
"""DistMult metapath scoring kernel for Trainium2 (8 NeuronCores).

Math (from the reference): every output element is emb_h[idx] @ c_g where
c_g = K_{g%2} @ s_g is one of six fixed [d] vectors, and each s_g is a sum
of embedding rows over an index list:
    pos0: emb_A[ei0[0]] . c0   c0 = K0 @ sum emb_A[ei0[1]]
    pos1: emb_A[ei1[0]] . c1   c1 = K1 @ sum emb_B[ei1[1]]
    nh0:  emb_A[nh0.flat] . c2 c2 = 16*K0 @ sum emb_A[nh0[:,0]]
    nh1:  emb_A[nh1.flat] . c3 c3 = 16*K1 @ sum emb_A[nh1[:,0]]
    nt0:  emb_A[nt0[:,0]] . c4 (x16)  c4 = K0 @ sum emb_A[nt0.flat]
    nt1:  emb_B[nt1[:,0]] . c5 (x16)  c5 = K1 @ sum emb_B[nt1.flat]

So instead of gathering ~450k embedding rows per core (the old kernel's
dma_gather bottleneck: GpSimd 97% busy generating descriptors), the device
computes per-NODE score tables with dense matmuls over table shards:
    P_A = emb_A @ [c0 c1 c2 c3 c4]   (each core: its 1/8 of the rows)
    P_B = emb_B @ [c5]
and the host assembles the per-edge outputs by pure indexing (same class of
host glue as the old kernel's take-maps / x16 expansion).

The row sums s_g become matmuls against histograms of the index lists
(h_g[n] = multiplicity of node n), computed host-side from the int index
tensors — index-derived preprocessing only, all float math stays on device.

Tables/histograms travel as bf16 (histogram counts are small ints = exact;
bf16 rounding of embeddings costs ~0.4% relerr against a 2e-2 tolerance)
which halves HBM traffic and runs the PE at full rate. PSUM accumulation
stays fp32.

Device pipeline per core (table-parallel over node rows):
  1. DMA in: table shard in two layouts (row-major wrapped for stage 1,
     transposed for stage 2), histograms, rel kernels.
  2. Stage 1: s^T partials = h^T @ shard per [128,128] tile (h as the tiny
     stationary operand so there are no big weight loads); one AllReduce of
     the [6,128] partials.
  3. Head: transpose s^T, c_g = K_{g%2} @ s_g on PE, x16 for nh.
  4. Stage 2: P^T chunks = C^T @ shard^T on PE (bf16, 512-wide into fp32
     PSUM), grouped copies alternating DVE/GpSimd, DMA out.
"""

import sys
from contextlib import ExitStack

import ml_dtypes
import numpy as np

sys.path.insert(0, "/opt/trn_rl_repo")

import concourse.bass as bass
from concourse import bacc, mybir
from concourse.bass_utils import run_bass_kernel_spmd
from concourse.masks import make_identity
from concourse.tile import TileContext

D = 128
E = 50000
S = 16
NA = 100000
NB = 50000
NCORES = 8

SH_A = NA // NCORES       # 12500 A-rows per core
SH_B = NB // NCORES       # 6250 B-rows per core
TA = (SH_A + 127) // 128  # 98 tiles
TB = (SH_B + 127) // 128  # 49 tiles
RA = TA * 128             # 12544 padded rows
RB = TB * 128             # 6272
GA = 4                    # A-table sum groups: [s0, h0, t0, h1]
GB = 2                    # B-table sum groups: [s1, t1]
CW = 512                  # stage-2 matmul width (one fp32 PSUM bank)
GW = 1024                 # stage-2 copy/DMA group width (two banks)
RM_CHUNKS_A = 4           # pipelined row-major loads
RM_CHUNKS_B = 2

F32 = mybir.dt.float32
BF16 = mybir.dt.bfloat16
NP_BF16 = ml_dtypes.bfloat16


def _chunks(total_tiles, n):
    base = total_tiles // n
    rem = total_tiles % n
    return [base + (1 if i < rem else 0) for i in range(n)]


def build() -> bass.Bass:
    nc = bacc.Bacc(None, target_bir_lowering=False)
    rmA = nc.dram_tensor("rmA", [128, RA], BF16, kind="ExternalInput")
    rmB = nc.dram_tensor("rmB", [128, RB], BF16, kind="ExternalInput")
    atA = nc.dram_tensor("atA", [128, RA], BF16, kind="ExternalInput")
    atB = nc.dram_tensor("atB", [128, RB], BF16, kind="ExternalInput")
    hA = nc.dram_tensor("hA", [128, TA * GA], BF16, kind="ExternalInput")
    hB = nc.dram_tensor("hB", [128, TB * GB], BF16, kind="ExternalInput")
    rel = nc.dram_tensor("rel", [2, D, D], F32, kind="ExternalInput")
    o_PA = nc.dram_tensor("o_PA", [5, RA], F32, kind="ExternalOutput")
    o_PB = nc.dram_tensor("o_PB", [1, RB], F32, kind="ExternalOutput")
    cc_in = nc.dram_tensor("cc_in", [128, 6], F32)
    cc_out = nc.dram_tensor("cc_out", [128, 6], F32, addr_space="Shared")

    with ExitStack() as ctx:
        tc = ctx.enter_context(TileContext(nc))
        sing = ctx.enter_context(tc.tile_pool(name="sing", bufs=1))
        scp = ctx.enter_context(tc.tile_pool(name="scp", bufs=3))
        ppK = ctx.enter_context(tc.tile_pool(name="ppK", bufs=1, space="PSUM"))
        ppS = ctx.enter_context(tc.tile_pool(name="ppS", bufs=1, space="PSUM"))
        ppP = ctx.enter_context(tc.tile_pool(name="ppP", bufs=2, space="PSUM"))

        ident = sing.tile([128, 128], F32)
        make_identity(nc, ident[:, :])

        # ---- input DMAs (sync queue, in priority order) ----
        kin = []
        for m in range(2):
            ki = sing.tile([128, 128], F32, tag=f"kin{m}")
            nc.sync.dma_start(out=ki[:, :], in_=rel[m, :, :])
            kin.append(ki)
        hA_t = sing.tile([128, TA * GA], BF16, tag="hA")
        nc.sync.dma_start(out=hA_t[:, :], in_=hA[:, :])
        hB_t = sing.tile([128, TB * GB], BF16, tag="hB")
        nc.sync.dma_start(out=hB_t[:, :], in_=hB[:, :])

        rmA_tiles = []  # (tile, ntiles)
        off = 0
        for i, nt in enumerate(_chunks(TA, RM_CHUNKS_A)):
            t = sing.tile([128, nt * 128], BF16, tag=f"rmA{i}")
            nc.sync.dma_start(out=t[:, :], in_=rmA[:, off : off + nt * 128])
            rmA_tiles.append((t, nt))
            off += nt * 128
        rmB_tiles = []
        off = 0
        for i, nt in enumerate(_chunks(TB, RM_CHUNKS_B)):
            t = sing.tile([128, nt * 128], BF16, tag=f"rmB{i}")
            nc.sync.dma_start(out=t[:, :], in_=rmB[:, off : off + nt * 128])
            rmB_tiles.append((t, nt))
            off += nt * 128

        atA_t = sing.tile([128, RA], BF16, tag="atA")
        nc.sync.dma_start(out=atA_t[:, :], in_=atA[:, :])
        atB_t = sing.tile([128, RB], BF16, tag="atB")
        nc.sync.dma_start(out=atB_t[:, :], in_=atB[:, :])

        # ---- K transposes (PE, early) ----
        KT = []
        for m in range(2):
            kt_ps = ppK.tile([128, 128], F32, tag="t")
            nc.tensor.transpose(out=kt_ps[:, :], in_=kin[m][:, :], identity=ident[:, :])
            kt = sing.tile([128, 128], F32, tag=f"kt{m}")
            nc.vector.tensor_copy(kt[:, :], kt_ps[:, :])
            KT.append(kt)

        # ---- stage 1: s^T partials = h^T @ shard, tile by tile ----
        # sT rows: [s0, h0, t0, h1] (A) and [s1, t1] (B); K0 -> 0:3, K1 -> 3:6
        sTA_sb = sing.tile([GA, 128], F32, tag="sTA_sb")
        sTB_sb = sing.tile([GB, 128], F32, tag="sTB_sb")
        sT_psA = ppS.tile([6, 128], F32, tag="s")
        gt = 0
        for t, nt in rmA_tiles:
            for i in range(nt):
                nc.tensor.matmul(
                    out=sT_psA[0:GA, :],
                    lhsT=hA_t[:, gt * GA : (gt + 1) * GA],
                    rhs=t[:, i * 128 : (i + 1) * 128],
                    start=(gt == 0),
                    stop=(gt == TA - 1),
                    skip_group_check=True,
                )
                gt += 1
        nc.vector.tensor_copy(sTA_sb[:, :], sT_psA[0:GA, :])
        sT_psB = ppS.tile([6, 128], F32, tag="s")  # same bank, WAR on copy
        gt = 0
        for t, nt in rmB_tiles:
            for i in range(nt):
                nc.tensor.matmul(
                    out=sT_psB[0:GB, :],
                    lhsT=hB_t[:, gt * GB : (gt + 1) * GB],
                    rhs=t[:, i * 128 : (i + 1) * 128],
                    start=(gt == 0),
                    stop=(gt == TB - 1),
                    skip_group_check=True,
                )
                gt += 1
        nc.vector.tensor_copy(sTB_sb[:, :], sT_psB[0:GB, :])
        # transpose BEFORE the collective (this side is overlapped; the
        # post-collective side is serial tail)
        s_ps = ppK.tile([128, 128], F32, tag="t")
        nc.tensor.matmul(
            out=s_ps[:, 0:GA], lhsT=sTA_sb[:, :], rhs=ident[:GA, :GA],
            is_transpose=True, skip_group_check=True,
        )
        nc.tensor.matmul(
            out=s_ps[:, GA : GA + GB], lhsT=sTB_sb[:, :], rhs=ident[:GB, :GB],
            is_transpose=True, skip_group_check=True,
        )
        s_cc = sing.tile([128, 6], F32, tag="s_cc")
        nc.vector.tensor_copy(s_cc[:, :], s_ps[:, :6])
        nc.scalar.dma_start(out=cc_in[:, :], in_=s_cc[:, :])
        nc.gpsimd.collective_compute(
            "AllReduce",
            mybir.AluOpType.add,
            replica_groups=[list(range(NCORES))],
            ins=[cc_in[:, :]],
            outs=[cc_out[:, :]],
        )
        s_sb = sing.tile([128, 6], F32, tag="s_sb")
        nc.scalar.dma_start(out=s_sb[:, :], in_=cc_out[:, :])

        # ---- head: c_ps cols = [K0s0, K0h0, K0t0, K1h1, K1s1, K1t1] ----
        c_ps = ppK.tile([128, 128], F32, tag="t")
        nc.tensor.matmul(
            out=c_ps[:, 0:3], lhsT=KT[0][:, :], rhs=s_sb[:, 0:3],
            start=True, stop=True, skip_group_check=True,
        )
        nc.tensor.matmul(
            out=c_ps[:, 3:6], lhsT=KT[1][:, :], rhs=s_sb[:, 3:6],
            start=True, stop=True, skip_group_check=True,
        )
        # C_A cols: [pos0=cp0, nh0=16*cp1, nt0=cp2, pos1=cp4, nh1=16*cp3]
        C_A = sing.tile([128, 5], BF16, tag="C_A")
        C_B = sing.tile([128, 1], BF16, tag="C_B")
        nc.vector.tensor_copy(C_A[:, 0:3], c_ps[:, 0:3])
        nc.vector.tensor_copy(C_A[:, 3:4], c_ps[:, 4:5])
        nc.vector.tensor_scalar_mul(C_A[:, 4:5], c_ps[:, 3:4], float(S))
        nc.vector.tensor_scalar_mul(C_A[:, 1:2], C_A[:, 1:2], float(S))
        nc.vector.tensor_copy(C_B[:, 0:1], c_ps[:, 5:6])

        # ---- stage 2: P^T = C^T @ shard^T, 512-wide matmuls, 1024-wide
        # copy/DMA groups alternating DVE / GpSimd ----
        copy_fns = [nc.vector.tensor_copy, nc.scalar.copy]
        gi = 0

        def stage2(C, ncols, at_t, width, o_dram):
            nonlocal gi
            for goff in range(0, width, GW):
                gw = min(GW, width - goff)
                pt = ppP.tile([5, GW], F32, tag="p")
                for coff in range(0, gw, CW):
                    w = min(CW, gw - coff)
                    nc.tensor.matmul(
                        out=pt[:ncols, coff : coff + w],
                        lhsT=C[:, :ncols],
                        rhs=at_t[:, goff + coff : goff + coff + w],
                        start=True,
                        stop=True,
                    )
                st = scp.tile([5, GW], F32, tag="st")
                copy_fns[gi % 2](st[:ncols, :gw], pt[:ncols, :gw])
                gi += 1
                nc.sync.dma_start(
                    out=o_dram[:, goff : goff + gw], in_=st[:ncols, :gw]
                )

        stage2(C_A, 5, atA_t, RA, o_PA)
        stage2(C_B, 1, atB_t, RB, o_PB)

    nc.compile()
    return nc


_CACHE = {}


def _program():
    if "p" not in _CACHE:
        _CACHE["p"] = build()
    return _CACHE["p"]


# ---------------------------------------------------------------- host glue


def _wrap_rm(shard, ntiles, ncols):
    """[ntiles*128, ncols] -> [128, ntiles*ncols] with [p, t*ncols+c] =
    shard[t*128+p, c] (tile-wrapped row-major layout)."""
    return np.ascontiguousarray(
        shard.reshape(ntiles, 128, ncols).transpose(1, 0, 2).reshape(128, -1)
    )


def kernel(
    emb_A,
    emb_B,
    rel_emb,
    edge_index_m0,
    edge_index_m1,
    neg_head_m0,
    neg_head_m1,
    neg_tail_m0,
    neg_tail_m1,
    _results=None,
):
    emb_A = np.asarray(emb_A, dtype=np.float32).astype(NP_BF16)
    emb_B = np.asarray(emb_B, dtype=np.float32).astype(NP_BF16)
    rel_emb = np.ascontiguousarray(np.asarray(rel_emb, dtype=np.float32))
    ei0 = np.asarray(edge_index_m0, dtype=np.int64)
    ei1 = np.asarray(edge_index_m1, dtype=np.int64)
    nh0 = np.asarray(neg_head_m0, dtype=np.int64)
    nh1 = np.asarray(neg_head_m1, dtype=np.int64)
    nt0 = np.asarray(neg_tail_m0, dtype=np.int64)
    nt1 = np.asarray(neg_tail_m1, dtype=np.int64)

    # index-derived histograms (multiplicity of each node in each sum group);
    # counts are small ints, exactly representable in bf16
    histA = np.zeros((NA, GA), np.float32)  # cols [s0, h0, t0, h1]
    histA[:, 0] = np.bincount(ei0[1], minlength=NA)
    histA[:, 1] = np.bincount(nh0[:, 0], minlength=NA)
    histA[:, 2] = np.bincount(nt0.reshape(-1), minlength=NA)
    histA[:, 3] = np.bincount(nh1[:, 0], minlength=NA)
    histB = np.zeros((NB, GB), np.float32)  # cols [s1, t1]
    histB[:, 0] = np.bincount(ei1[1], minlength=NB)
    histB[:, 1] = np.bincount(nt1.reshape(-1), minlength=NB)

    in_maps = []
    for k in range(NCORES):
        a_sh = np.zeros((RA, D), NP_BF16)
        a_sh[:SH_A] = emb_A[k * SH_A : (k + 1) * SH_A]
        b_sh = np.zeros((RB, D), NP_BF16)
        b_sh[:SH_B] = emb_B[k * SH_B : (k + 1) * SH_B]
        ha = np.zeros((RA, GA), NP_BF16)
        ha[:SH_A] = histA[k * SH_A : (k + 1) * SH_A]
        hb = np.zeros((RB, GB), NP_BF16)
        hb[:SH_B] = histB[k * SH_B : (k + 1) * SH_B]
        in_maps.append(
            {
                "rmA": _wrap_rm(a_sh, TA, D),
                "rmB": _wrap_rm(b_sh, TB, D),
                "atA": np.ascontiguousarray(a_sh.T),
                "atB": np.ascontiguousarray(b_sh.T),
                "hA": _wrap_rm(ha, TA, GA),
                "hB": _wrap_rm(hb, TB, GB),
                "rel": rel_emb,
            }
        )

    prog = _program()
    cores = list(range(NCORES))
    r = run_bass_kernel_spmd(prog, in_maps, cores)
    if _results is not None:
        _results.append(r)

    # per-node score tables: PA cols [pos0, nh0, nt0, pos1, nh1]; PB [nt1]
    PA = np.empty((NA, 5), np.float32)
    PB = np.empty((NB, 1), np.float32)
    for k in cores:
        PA[k * SH_A : (k + 1) * SH_A] = r.results[k]["o_PA"][:, :SH_A].T
        PB[k * SH_B : (k + 1) * SH_B] = r.results[k]["o_PB"][:, :SH_B].T

    pos0 = PA[ei0[0], 0]
    pos1 = PA[ei1[0], 3]
    nh0_s = PA[nh0.reshape(-1), 1]
    nh1_s = PA[nh1.reshape(-1), 4]
    nt0_s = np.repeat(PA[nt0[:, 0], 2], S)
    nt1_s = np.repeat(PB[nt1[:, 0], 0], S)
    return np.concatenate([pos0, pos1, nh0_s, nh1_s, nt0_s, nt1_s])


# revision 22
# speedup vs baseline: 37.8791x; 1.2066x over previous
"""DistMult metapath scoring kernel for Trainium2 (8 NeuronCores).

Math (from the reference): every output element is emb_h[idx] @ c_g where
c_g = K_{g%2} @ s_g is one of six fixed [d] vectors, and each s_g is a sum
of embedding rows over an index list:
    pos0: emb_A[ei0[0]] . c0   c0 = K0 @ sum emb_A[ei0[1]]
    pos1: emb_A[ei1[0]] . c1   c1 = K1 @ sum emb_B[ei1[1]]
    nh0:  emb_A[nh0.flat] . c2 c2 = 16*K0 @ sum emb_A[nh0[:,0]]
    nh1:  emb_A[nh1.flat] . c3 c3 = 16*K1 @ sum emb_A[nh1[:,0]]
    nt0:  emb_A[nt0[:,0]] . c4 (x16)  c4 = K0 @ sum emb_A[nt0.flat]
    nt1:  emb_B[nt1[:,0]] . c5 (x16)  c5 = K1 @ sum emb_B[nt1.flat]

So instead of gathering ~450k embedding rows per core (the old kernel's
dma_gather bottleneck: GpSimd 97% busy generating descriptors), the device
computes per-NODE score tables with dense matmuls over table shards:
    P_A = emb_A @ [c0 c1 c2 c3 c4]   (each core: its 1/8 of the rows)
    P_B = emb_B @ [c5]
and the host assembles the per-edge outputs by pure indexing (same class of
host glue as the old kernel's take-maps / x16 expansion).

The row sums s_g become matmuls against histograms of the index lists
(h_g[n] = multiplicity of node n), computed host-side from the int index
tensors — index-derived preprocessing only, all float math stays on device.

Tables/histograms travel as bf16 (histogram counts are small ints = exact;
bf16 rounding of embeddings costs ~0.4% relerr against a 2e-2 tolerance)
which halves HBM traffic and runs the PE at full rate. PSUM accumulation
stays fp32.

Device pipeline per core (table-parallel over node rows):
  1. DMA in: table shard in two layouts (row-major wrapped for stage 1,
     transposed for stage 2), histograms, rel kernels.
  2. Stage 1: s^T partials = h^T @ shard per [128,128] tile (h as the tiny
     stationary operand so there are no big weight loads); one AllReduce of
     the [6,128] partials.
  3. Head: transpose s^T, c_g = K_{g%2} @ s_g on PE, x16 for nh.
  4. Stage 2: P^T chunks = C^T @ shard^T on PE (bf16, 512-wide into fp32
     PSUM), grouped copies alternating DVE/GpSimd, DMA out.
"""

import sys
from contextlib import ExitStack

import ml_dtypes
import numpy as np

sys.path.insert(0, "/opt/trn_rl_repo")

import concourse.bass as bass
from concourse import bacc, mybir
from concourse.bass_utils import run_bass_kernel_spmd
from concourse.masks import make_identity
from concourse.tile import TileContext

D = 128
E = 50000
S = 16
NA = 100000
NB = 50000
NCORES = 8

SH_A = NA // NCORES       # 12500 A-rows per core
SH_B = NB // NCORES       # 6250 B-rows per core
TA = (SH_A + 127) // 128  # 98 tiles
TB = (SH_B + 127) // 128  # 49 tiles
RA = TA * 128             # 12544 padded rows
RB = TB * 128             # 6272
GA = 4                    # A-table sum groups: [s0, h0, t0, h1]
GB = 2                    # B-table sum groups: [s1, t1]
CW = 512                  # stage-2 matmul width (one fp32 PSUM bank)
GW = 1024                 # stage-2 copy/DMA group width (two banks)
RM_CHUNKS_A = 4           # pipelined row-major loads
RM_CHUNKS_B = 2

F32 = mybir.dt.float32
BF16 = mybir.dt.bfloat16
NP_BF16 = ml_dtypes.bfloat16


def _chunks(total_tiles, n):
    base = total_tiles // n
    rem = total_tiles % n
    return [base + (1 if i < rem else 0) for i in range(n)]


def build() -> bass.Bass:
    nc = bacc.Bacc(None, target_bir_lowering=False)
    rmA = nc.dram_tensor("rmA", [128, RA], BF16, kind="ExternalInput")
    rmB = nc.dram_tensor("rmB", [128, RB], BF16, kind="ExternalInput")
    hA = nc.dram_tensor("hA", [128, TA * GA], BF16, kind="ExternalInput")
    hB = nc.dram_tensor("hB", [128, TB * GB], BF16, kind="ExternalInput")
    rel = nc.dram_tensor("rel", [2, D, D], F32, kind="ExternalInput")
    o_PA = nc.dram_tensor("o_PA", [5, RA], F32, kind="ExternalOutput")
    o_PB = nc.dram_tensor("o_PB", [1, RB], F32, kind="ExternalOutput")
    cc_in = nc.dram_tensor("cc_in", [128, 6], F32)
    cc_out = nc.dram_tensor("cc_out", [128, 6], F32, addr_space="Shared")

    with ExitStack() as ctx:
        tc = ctx.enter_context(TileContext(nc))
        sing = ctx.enter_context(tc.tile_pool(name="sing", bufs=1))
        scp = ctx.enter_context(tc.tile_pool(name="scp", bufs=6))
        ppK = ctx.enter_context(tc.tile_pool(name="ppK", bufs=1, space="PSUM"))
        ppS = ctx.enter_context(tc.tile_pool(name="ppS", bufs=1, space="PSUM"))
        ppP = ctx.enter_context(tc.tile_pool(name="ppP", bufs=2, space="PSUM"))
        ppT = ctx.enter_context(tc.tile_pool(name="ppT", bufs=2, space="PSUM"))

        ident = sing.tile([128, 128], F32)
        make_identity(nc, ident[:, :])

        # ---- input DMAs (sync queue, in priority order) ----
        kin = []
        for m in range(2):
            ki = sing.tile([128, 128], F32, tag=f"kin{m}")
            nc.sync.dma_start(out=ki[:, :], in_=rel[m, :, :])
            kin.append(ki)
        hA_t = sing.tile([128, TA * GA], BF16, tag="hA")
        nc.sync.dma_start(out=hA_t[:, :], in_=hA[:, :])
        hB_t = sing.tile([128, TB * GB], BF16, tag="hB")
        nc.sync.dma_start(out=hB_t[:, :], in_=hB[:, :])

        rmA_tiles = []  # (tile, ntiles)
        off = 0
        for i, nt in enumerate(_chunks(TA, RM_CHUNKS_A)):
            t = sing.tile([128, nt * 128], BF16, tag=f"rmA{i}")
            nc.sync.dma_start(out=t[:, :], in_=rmA[:, off : off + nt * 128])
            rmA_tiles.append((t, nt))
            off += nt * 128
        rmB_tiles = []
        off = 0
        for i, nt in enumerate(_chunks(TB, RM_CHUNKS_B)):
            t = sing.tile([128, nt * 128], BF16, tag=f"rmB{i}")
            nc.sync.dma_start(out=t[:, :], in_=rmB[:, off : off + nt * 128])
            rmB_tiles.append((t, nt))
            off += nt * 128

        # transposed tables are built ON-CHIP from the rm tiles (PE is idle
        # while the collective runs), not DMA'd — halves input HBM traffic
        atA_t = sing.tile([128, RA], BF16, tag="atA")
        atB_t = sing.tile([128, RB], BF16, tag="atB")
        ident_bf = sing.tile([128, 128], BF16, tag="ident_bf")
        nc.vector.tensor_copy(ident_bf[:, :], ident[:, :])

        # ---- K transposes (PE, early) ----
        KT = []
        for m in range(2):
            kt_ps = ppK.tile([128, 128], F32, tag="t")
            nc.tensor.transpose(out=kt_ps[:, :], in_=kin[m][:, :], identity=ident[:, :])
            kt = sing.tile([128, 128], F32, tag=f"kt{m}")
            nc.vector.tensor_copy(kt[:, :], kt_ps[:, :])
            KT.append(kt)

        # ---- stage 1: s^T partials = h^T @ shard, tile by tile ----
        # sT rows: [s0, h0, t0, h1] (A) and [s1, t1] (B); K0 -> 0:3, K1 -> 3:6
        sTA_sb = sing.tile([GA, 128], F32, tag="sTA_sb")
        sTB_sb = sing.tile([GB, 128], F32, tag="sTB_sb")
        sT_psA = ppS.tile([6, 128], F32, tag="s")
        gt = 0
        for t, nt in rmA_tiles:
            for i in range(nt):
                nc.tensor.matmul(
                    out=sT_psA[0:GA, :],
                    lhsT=hA_t[:, gt * GA : (gt + 1) * GA],
                    rhs=t[:, i * 128 : (i + 1) * 128],
                    start=(gt == 0),
                    stop=(gt == TA - 1),
                    skip_group_check=True,
                )
                gt += 1
        nc.vector.tensor_copy(sTA_sb[:, :], sT_psA[0:GA, :])
        sT_psB = ppS.tile([6, 128], F32, tag="s")  # same bank, WAR on copy
        gt = 0
        for t, nt in rmB_tiles:
            for i in range(nt):
                nc.tensor.matmul(
                    out=sT_psB[0:GB, :],
                    lhsT=hB_t[:, gt * GB : (gt + 1) * GB],
                    rhs=t[:, i * 128 : (i + 1) * 128],
                    start=(gt == 0),
                    stop=(gt == TB - 1),
                    skip_group_check=True,
                )
                gt += 1
        nc.vector.tensor_copy(sTB_sb[:, :], sT_psB[0:GB, :])
        # transpose BEFORE the collective (this side is overlapped; the
        # post-collective side is serial tail)
        s_ps = ppK.tile([128, 128], F32, tag="t")
        nc.tensor.matmul(
            out=s_ps[:, 0:GA], lhsT=sTA_sb[:, :], rhs=ident[:GA, :GA],
            is_transpose=True, skip_group_check=True,
        )
        nc.tensor.matmul(
            out=s_ps[:, GA : GA + GB], lhsT=sTB_sb[:, :], rhs=ident[:GB, :GB],
            is_transpose=True, skip_group_check=True,
        )
        s_cc = sing.tile([128, 6], F32, tag="s_cc")
        nc.vector.tensor_copy(s_cc[:, :], s_ps[:, :6])
        nc.gpsimd.dma_start(out=cc_in[:, :], in_=s_cc[:, :])
        nc.gpsimd.collective_compute(
            "AllReduce",
            mybir.AluOpType.add,
            replica_groups=[list(range(NCORES))],
            ins=[cc_in[:, :]],
            outs=[cc_out[:, :]],
        )
        s_sb = sing.tile([128, 6], F32, tag="s_sb")
        nc.gpsimd.dma_start(out=s_sb[:, :], in_=cc_out[:, :])

        # ---- on-chip transposes: at[:, t*128:(t+1)*128] = rm_tile^T ----
        copy_fns = [nc.vector.tensor_copy, nc.scalar.copy]
        ti = 0
        for at_t, tiles in ((atA_t, rmA_tiles), (atB_t, rmB_tiles)):
            goff = 0
            for t, nt in tiles:
                for i in range(nt):
                    tp = ppT.tile([128, 128], BF16, tag="tp")
                    nc.tensor.matmul(
                        out=tp[:, :],
                        lhsT=t[:, i * 128 : (i + 1) * 128],
                        rhs=ident_bf[:, :],
                        is_transpose=True,
                        skip_group_check=True,
                    )
                    copy_fns[ti % 2](
                        at_t[:, goff : goff + 128], tp[:, :]
                    )
                    ti += 1
                    goff += 128

        # ---- head: c_ps cols = [K0s0, K0h0, K0t0, K1h1, K1s1, K1t1] ----
        c_ps = ppK.tile([128, 128], F32, tag="t")
        nc.tensor.matmul(
            out=c_ps[:, 0:3], lhsT=KT[0][:, :], rhs=s_sb[:, 0:3],
            start=True, stop=True, skip_group_check=True,
        )
        nc.tensor.matmul(
            out=c_ps[:, 3:6], lhsT=KT[1][:, :], rhs=s_sb[:, 3:6],
            start=True, stop=True, skip_group_check=True,
        )
        # C_A cols: [pos0=cp0, nh0=16*cp1, nt0=cp2, pos1=cp4, nh1=16*cp3]
        C_A = sing.tile([128, 5], BF16, tag="C_A")
        C_B = sing.tile([128, 1], BF16, tag="C_B")
        nc.vector.tensor_copy(C_A[:, 0:3], c_ps[:, 0:3])
        nc.vector.tensor_copy(C_A[:, 3:4], c_ps[:, 4:5])
        nc.vector.tensor_scalar_mul(C_A[:, 4:5], c_ps[:, 3:4], float(S))
        nc.vector.tensor_scalar_mul(C_A[:, 1:2], C_A[:, 1:2], float(S))
        nc.vector.tensor_copy(C_B[:, 0:1], c_ps[:, 5:6])

        # ---- stage 2: P^T = C^T @ shard^T, 512-wide matmuls, 1024-wide
        # copy/DMA groups alternating DVE / GpSimd ----
        gi = 0

        def stage2(C, ncols, at_t, width, o_dram):
            nonlocal gi
            for goff in range(0, width, GW):
                gw = min(GW, width - goff)
                pt = ppP.tile([5, GW], F32, tag="p")
                for coff in range(0, gw, CW):
                    w = min(CW, gw - coff)
                    nc.tensor.matmul(
                        out=pt[:ncols, coff : coff + w],
                        lhsT=C[:, :ncols],
                        rhs=at_t[:, goff + coff : goff + coff + w],
                        start=True,
                        stop=True,
                    )
                st = scp.tile([5, GW], F32, tag="st")
                copy_fns[gi % 2](st[:ncols, :gw], pt[:ncols, :gw])
                gi += 1
                nc.sync.dma_start(
                    out=o_dram[:, goff : goff + gw], in_=st[:ncols, :gw]
                )

        stage2(C_A, 5, atA_t, RA, o_PA)
        stage2(C_B, 1, atB_t, RB, o_PB)

    nc.compile()
    return nc


_CACHE = {}


def _program():
    if "p" not in _CACHE:
        _CACHE["p"] = build()
    return _CACHE["p"]


# ---------------------------------------------------------------- host glue


def _wrap_rm(shard, ntiles, ncols):
    """[ntiles*128, ncols] -> [128, ntiles*ncols] with [p, t*ncols+c] =
    shard[t*128+p, c] (tile-wrapped row-major layout)."""
    return np.ascontiguousarray(
        shard.reshape(ntiles, 128, ncols).transpose(1, 0, 2).reshape(128, -1)
    )


def kernel(
    emb_A,
    emb_B,
    rel_emb,
    edge_index_m0,
    edge_index_m1,
    neg_head_m0,
    neg_head_m1,
    neg_tail_m0,
    neg_tail_m1,
    _results=None,
):
    emb_A = np.asarray(emb_A, dtype=np.float32).astype(NP_BF16)
    emb_B = np.asarray(emb_B, dtype=np.float32).astype(NP_BF16)
    rel_emb = np.ascontiguousarray(np.asarray(rel_emb, dtype=np.float32))
    ei0 = np.asarray(edge_index_m0, dtype=np.int64)
    ei1 = np.asarray(edge_index_m1, dtype=np.int64)
    nh0 = np.asarray(neg_head_m0, dtype=np.int64)
    nh1 = np.asarray(neg_head_m1, dtype=np.int64)
    nt0 = np.asarray(neg_tail_m0, dtype=np.int64)
    nt1 = np.asarray(neg_tail_m1, dtype=np.int64)

    # index-derived histograms (multiplicity of each node in each sum group);
    # counts are small ints, exactly representable in bf16
    histA = np.zeros((NA, GA), np.float32)  # cols [s0, h0, t0, h1]
    histA[:, 0] = np.bincount(ei0[1], minlength=NA)
    histA[:, 1] = np.bincount(nh0[:, 0], minlength=NA)
    histA[:, 2] = np.bincount(nt0.reshape(-1), minlength=NA)
    histA[:, 3] = np.bincount(nh1[:, 0], minlength=NA)
    histB = np.zeros((NB, GB), np.float32)  # cols [s1, t1]
    histB[:, 0] = np.bincount(ei1[1], minlength=NB)
    histB[:, 1] = np.bincount(nt1.reshape(-1), minlength=NB)

    in_maps = []
    for k in range(NCORES):
        a_sh = np.zeros((RA, D), NP_BF16)
        a_sh[:SH_A] = emb_A[k * SH_A : (k + 1) * SH_A]
        b_sh = np.zeros((RB, D), NP_BF16)
        b_sh[:SH_B] = emb_B[k * SH_B : (k + 1) * SH_B]
        ha = np.zeros((RA, GA), NP_BF16)
        ha[:SH_A] = histA[k * SH_A : (k + 1) * SH_A]
        hb = np.zeros((RB, GB), NP_BF16)
        hb[:SH_B] = histB[k * SH_B : (k + 1) * SH_B]
        in_maps.append(
            {
                "rmA": _wrap_rm(a_sh, TA, D),
                "rmB": _wrap_rm(b_sh, TB, D),
                "hA": _wrap_rm(ha, TA, GA),
                "hB": _wrap_rm(hb, TB, GB),
                "rel": rel_emb,
            }
        )

    prog = _program()
    cores = list(range(NCORES))
    r = run_bass_kernel_spmd(prog, in_maps, cores)
    if _results is not None:
        _results.append(r)

    # per-node score tables: PA cols [pos0, nh0, nt0, pos1, nh1]; PB [nt1]
    PA = np.empty((NA, 5), np.float32)
    PB = np.empty((NB, 1), np.float32)
    for k in cores:
        PA[k * SH_A : (k + 1) * SH_A] = r.results[k]["o_PA"][:, :SH_A].T
        PB[k * SH_B : (k + 1) * SH_B] = r.results[k]["o_PB"][:, :SH_B].T

    pos0 = PA[ei0[0], 0]
    pos1 = PA[ei1[0], 3]
    nh0_s = PA[nh0.reshape(-1), 1]
    nh1_s = PA[nh1.reshape(-1), 4]
    nt0_s = np.repeat(PA[nt0[:, 0], 2], S)
    nt1_s = np.repeat(PB[nt1[:, 0], 0], S)
    return np.concatenate([pos0, pos1, nh0_s, nh1_s, nt0_s, nt1_s])
